# revision 1
# baseline (speedup 1.0000x reference)
"""Mixture-of-Experts (top-2 of 8 experts) Trainium2 kernel, 8 NeuronCores.

Strategy (expert-parallel, per sharding hint):
  - Router gate is tiny (T x 1024 @ 1024 x 8) and determines the sharding
    itself, so it is evaluated on the host (in float64) as part of the
    dispatch step; each token is sent to its top-2 experts.
  - Expert e's FFN runs entirely on core e: tokens routed to expert e are
    gathered, padded to a common capacity C, and the dense
    gelu(x @ w1 + b1) @ w2 FFN runs on that core in bf16 with fp32
    accumulation (TensorE native rate).
  - The combine step (scale by softmax weight, add b2, scatter-add over the
    two expert contributions per token) runs on the host.

Device layout per core (expert e = core id):
  xT  [8, 128, C]   bf16  gathered tokens, transposed: xT[ko,p,t] = x_t[ko*128+p]
  w1  [8, 128, 4096] bf16  w1[e] with D split into 8 partition chunks
  w2  [32, 128, 1024] bf16 w2[e] with F split into 32 partition chunks
  b1  [128, 32]     f32   b1[e] chunked per partition
  yT  [8, 128, C]   f32   (gelu(x@w1+b1) @ w2)^T, combine applied on host
"""

import math

import ml_dtypes
import numpy as np

N_CORES = 8
D = 1024
F = 4096
E = 8
TOP_K = 2
KO = D // 128   # 8 partition chunks of D
FO = F // 128   # 32 partition chunks of F
CT = 512        # token tile (matmul moving dim)

BF16 = ml_dtypes.bfloat16

# Cache of compiled Bass modules keyed by token capacity C.
_NC_CACHE: dict[int, object] = {}

# Most recent BassKernelResults — exposed for the test harness (profiling).
LAST_RESULTS = None


def _token_tiles(C):
    """Split capacity C (multiple of 128) into matmul-friendly tiles."""
    tiles = []
    off = 0
    while off < C:
        w = min(CT, C - off)
        tiles.append((off, w))
        off += w
    return tiles


def _build(C):
    import concourse.mybir as mybir
    from concourse import bacc
    from concourse.tile import TileContext

    fp32 = mybir.dt.float32
    bf16 = mybir.dt.bfloat16

    nc = bacc.Bacc(
        "TRN2", target_bir_lowering=False, debug=False, num_devices=N_CORES
    )
    xT = nc.dram_tensor("xT", [KO, 128, C], bf16, kind="ExternalInput")
    w1 = nc.dram_tensor("w1", [KO, 128, F], bf16, kind="ExternalInput")
    w2 = nc.dram_tensor("w2", [FO, 128, D], bf16, kind="ExternalInput")
    b1 = nc.dram_tensor("b1", [128, FO], fp32, kind="ExternalInput")
    yT = nc.dram_tensor("yT", [KO, 128, C], fp32, kind="ExternalOutput")

    with TileContext(nc) as tc:
        with (
            tc.tile_pool(name="wpool", bufs=1) as wpool,
            tc.tile_pool(name="xpool", bufs=2) as xpool,
            tc.tile_pool(name="hpool", bufs=1) as hpool,
            tc.tile_pool(name="ypool", bufs=4) as ypool,
            tc.tile_pool(name="ph", bufs=2, space="PSUM") as phpool,
            tc.tile_pool(name="py", bufs=2, space="PSUM") as pypool,
        ):
            w1_sb = wpool.tile([128, KO, F], bf16)
            w2_sb = wpool.tile([128, FO, D], bf16)
            b1_sb = wpool.tile([128, FO], fp32)
            for ko in range(KO):
                nc.sync.dma_start(w1_sb[:, ko], w1[ko])
            for fo in range(FO):
                nc.sync.dma_start(w2_sb[:, fo], w2[fo])
            nc.sync.dma_start(b1_sb[:], b1[:])

            for off, tw in _token_tiles(C):
                x_sb = xpool.tile([128, KO, CT], bf16)
                for ko in range(KO):
                    nc.sync.dma_start(
                        x_sb[:, ko, :tw], xT[ko, :, off : off + tw]
                    )
                h_sb = hpool.tile([128, FO, CT], bf16)
                for fo in range(FO):
                    ph = phpool.tile([128, CT], fp32)
                    for ko in range(KO):
                        nc.tensor.matmul(
                            ph[:, :tw],
                            lhsT=w1_sb[:, ko, fo * 128 : (fo + 1) * 128],
                            rhs=x_sb[:, ko, :tw],
                            start=(ko == 0),
                            stop=(ko == KO - 1),
                        )
                    nc.scalar.activation(
                        h_sb[:, fo, :tw],
                        ph[:, :tw],
                        mybir.ActivationFunctionType.Gelu,
                        bias=b1_sb[:, fo : fo + 1],
                    )
                for do in range(KO):
                    py = pypool.tile([128, CT], fp32)
                    for fo in range(FO):
                        nc.tensor.matmul(
                            py[:, :tw],
                            lhsT=w2_sb[:, fo, do * 128 : (do + 1) * 128],
                            rhs=h_sb[:, fo, :tw],
                            start=(fo == 0),
                            stop=(fo == FO - 1),
                        )
                    y_sb = ypool.tile([128, CT], fp32)
                    nc.vector.tensor_copy(y_sb[:, :tw], py[:, :tw])
                    nc.sync.dma_start(yT[do, :, off : off + tw], y_sb[:, :tw])

    nc.compile()
    return nc


def kernel(x, gate_w, w1, b1, w2, b2):
    from concourse.bass_utils import run_bass_kernel_spmd

    global LAST_RESULTS

    x = np.asarray(x, dtype=np.float32)
    gate_w = np.asarray(gate_w, dtype=np.float32)
    w1 = np.asarray(w1, dtype=np.float32)
    b1 = np.asarray(b1, dtype=np.float32)
    w2 = np.asarray(w2, dtype=np.float32)
    b2 = np.asarray(b2, dtype=np.float32)

    B, S, Din = x.shape
    assert Din == D and gate_w.shape == (D, E)
    T = B * S
    xf = x.reshape(T, D)

    # ---- Host router (replicated gate): logits, top-2, softmax weights ----
    logits = xf.astype(np.float64) @ gate_w.astype(np.float64)  # [T, E]
    idx0 = np.argmax(logits, axis=1)
    rows = np.arange(T)
    v0 = logits[rows, idx0]
    l2 = logits.copy()
    l2[rows, idx0] = -np.inf
    idx1 = np.argmax(l2, axis=1)
    v1 = l2[rows, idx1]
    # softmax over the two top logits
    e1 = np.exp(v1 - v0)
    cw0 = 1.0 / (1.0 + e1)
    cw1 = e1 / (1.0 + e1)

    # ---- Dispatch: gather token ids per expert ----
    token_ids = []
    combine_w = []
    for e in range(E):
        sel0 = idx0 == e
        sel1 = idx1 == e
        ids = np.nonzero(sel0 | sel1)[0]
        w = np.where(sel0[ids], cw0[ids], cw1[ids])
        token_ids.append(ids)
        combine_w.append(w)

    max_n = max(len(ids) for ids in token_ids)
    C = max(CT, int(math.ceil(max_n / 128.0)) * 128)

    if C not in _NC_CACHE:
        _NC_CACHE[C] = _build(C)
    nc = _NC_CACHE[C]

    # ---- Build per-core input maps ----
    in_maps = []
    for e in range(E):
        ids = token_ids[e]
        n_e = len(ids)
        xT = np.zeros((KO, 128, C), dtype=BF16)
        if n_e:
            # [n_e, D] -> [D, n_e] -> [KO, 128, n_e]
            xg = xf[ids].T.reshape(KO, 128, n_e)
            xT[:, :, :n_e] = xg.astype(BF16)
        in_maps.append(
            {
                "xT": xT,
                "w1": np.ascontiguousarray(
                    w1[e].reshape(KO, 128, F).astype(BF16)
                ),
                "w2": np.ascontiguousarray(
                    w2[e].reshape(FO, 128, D).astype(BF16)
                ),
                "b1": np.ascontiguousarray(b1[e].reshape(FO, 128).T),
            }
        )

    res = run_bass_kernel_spmd(nc, in_maps, core_ids=list(range(N_CORES)))
    LAST_RESULTS = res

    # ---- Combine on host: out[t] += cw * (y_e[t] + b2[e]) ----
    out = np.zeros((T, D), dtype=np.float32)
    for e in range(E):
        ids = token_ids[e]
        n_e = len(ids)
        if n_e == 0:
            continue
        y_t = res.results[e]["yT"].reshape(D, C)[:, :n_e].T  # [n_e, D]
        out[ids] += combine_w[e][:, None].astype(np.float32) * (y_t + b2[e])

    return out.reshape(B, S, D)


# revision 2
# speedup vs baseline: 1.0795x; 1.0795x over previous
"""Mixture-of-Experts (top-2 of 8 experts) Trainium2 kernel, 8 NeuronCores.

Strategy (expert-parallel, per sharding hint):
  - Router gate is tiny (T x 1024 @ 1024 x 8) and determines the sharding
    itself, so it is evaluated on the host (in float64) as part of the
    dispatch step; each token is sent to its top-2 experts.
  - Expert e's FFN runs entirely on core e: tokens routed to expert e are
    gathered, padded to a common capacity C, and the dense
    gelu(x @ w1 + b1) @ w2 FFN runs on that core in bf16 with fp32
    accumulation (TensorE native rate).
  - The combine step (scale by softmax weight, add b2, scatter-add over the
    two expert contributions per token) runs on the host.

Device layout per core (expert e = core id):
  xT  [8, 128, C]   bf16  gathered tokens, transposed: xT[ko,p,t] = x_t[ko*128+p]
  w1  [8, 128, 4096] bf16  w1[e] with D split into 8 partition chunks
  w2  [32, 128, 1024] bf16 w2[e] with F split into 32 partition chunks
  b1  [128, 32]     f32   b1[e] chunked per partition
  yT  [8, 128, C]   f32   (gelu(x@w1+b1) @ w2)^T, combine applied on host
"""

import math

import ml_dtypes
import numpy as np

N_CORES = 8
D = 1024
F = 4096
E = 8
TOP_K = 2
KO = D // 128   # 8 partition chunks of D
FO = F // 128   # 32 partition chunks of F
CT = 512        # token tile (matmul moving dim)

BF16 = ml_dtypes.bfloat16

# Cache of compiled Bass modules keyed by token capacity C.
_NC_CACHE: dict[int, object] = {}

# Most recent BassKernelResults — exposed for the test harness (profiling).
LAST_RESULTS = None


def _token_tiles(C):
    """Split capacity C (multiple of 128) into matmul-friendly tiles."""
    tiles = []
    off = 0
    while off < C:
        w = min(CT, C - off)
        tiles.append((off, w))
        off += w
    return tiles


def _build(C):
    import concourse.mybir as mybir
    from concourse import bacc
    from concourse.tile import TileContext

    fp32 = mybir.dt.float32
    bf16 = mybir.dt.bfloat16

    nc = bacc.Bacc(
        "TRN2", target_bir_lowering=False, debug=False, num_devices=N_CORES
    )
    xT = nc.dram_tensor("xT", [KO, 128, C], bf16, kind="ExternalInput")
    w1 = nc.dram_tensor("w1", [KO, 128, F], bf16, kind="ExternalInput")
    w2 = nc.dram_tensor("w2", [FO, 128, D], bf16, kind="ExternalInput")
    b1 = nc.dram_tensor("b1", [128, FO], fp32, kind="ExternalInput")
    yT = nc.dram_tensor("yT", [KO, 128, C], fp32, kind="ExternalOutput")

    with TileContext(nc) as tc:
        with (
            tc.tile_pool(name="wpool", bufs=1) as wpool,
            tc.tile_pool(name="xpool", bufs=2) as xpool,
            tc.tile_pool(name="hpool", bufs=1) as hpool,
            tc.tile_pool(name="ypool", bufs=4) as ypool,
            tc.tile_pool(name="ph", bufs=2, space="PSUM") as phpool,
            tc.tile_pool(name="py", bufs=2, space="PSUM") as pypool,
        ):
            w1_sb = wpool.tile([128, KO, F], bf16)
            w2_sb = wpool.tile([128, FO, D], bf16)
            b1_sb = wpool.tile([128, FO], fp32)
            nc.sync.dma_start(b1_sb[:], b1[:])

            tiles = _token_tiles(C)

            # First token tile's activations go first so the PE can start
            # as soon as the first quarter of w1 lands.
            x_first = xpool.tile([128, KO, CT], bf16, tag="x_sb")
            for ko in range(KO):
                nc.sync.dma_start(
                    x_first[:, ko, : tiles[0][1]], xT[ko, :, : tiles[0][1]]
                )
            # w1 split into F-quarters: the f-loop consumes quarter q after
            # only its 8 chunk DMAs (2 MB) instead of the full 8 MB.
            FQ = F // 4
            for q in range(4):
                for ko in range(KO):
                    nc.sync.dma_start(
                        w1_sb[:, ko, q * FQ : (q + 1) * FQ],
                        w1[ko, :, q * FQ : (q + 1) * FQ],
                    )
            # w2 is only needed once the first tile's gelu output exists
            # (~80us in) — issue after w1 so it doesn't steal HBM bandwidth.
            for fo in range(FO):
                nc.sync.dma_start(w2_sb[:, fo], w2[fo])

            for ti, (off, tw) in enumerate(tiles):
                if ti == 0:
                    x_sb = x_first
                else:
                    x_sb = xpool.tile([128, KO, CT], bf16, tag="x_sb")
                    for ko in range(KO):
                        nc.sync.dma_start(
                            x_sb[:, ko, :tw], xT[ko, :, off : off + tw]
                        )
                h_sb = hpool.tile([128, FO, CT], bf16)
                for fo in range(FO):
                    ph = phpool.tile([128, CT], fp32)
                    for ko in range(KO):
                        nc.tensor.matmul(
                            ph[:, :tw],
                            lhsT=w1_sb[:, ko, fo * 128 : (fo + 1) * 128],
                            rhs=x_sb[:, ko, :tw],
                            start=(ko == 0),
                            stop=(ko == KO - 1),
                        )
                    nc.scalar.activation(
                        h_sb[:, fo, :tw],
                        ph[:, :tw],
                        mybir.ActivationFunctionType.Gelu,
                        bias=b1_sb[:, fo : fo + 1],
                    )
                for do in range(KO):
                    py = pypool.tile([128, CT], fp32)
                    for fo in range(FO):
                        nc.tensor.matmul(
                            py[:, :tw],
                            lhsT=w2_sb[:, fo, do * 128 : (do + 1) * 128],
                            rhs=h_sb[:, fo, :tw],
                            start=(fo == 0),
                            stop=(fo == FO - 1),
                        )
                    y_sb = ypool.tile([128, CT], fp32)
                    nc.vector.tensor_copy(y_sb[:, :tw], py[:, :tw])
                    nc.sync.dma_start(yT[do, :, off : off + tw], y_sb[:, :tw])

    nc.compile()
    return nc


def kernel(x, gate_w, w1, b1, w2, b2):
    from concourse.bass_utils import run_bass_kernel_spmd

    global LAST_RESULTS

    x = np.asarray(x, dtype=np.float32)
    gate_w = np.asarray(gate_w, dtype=np.float32)
    w1 = np.asarray(w1, dtype=np.float32)
    b1 = np.asarray(b1, dtype=np.float32)
    w2 = np.asarray(w2, dtype=np.float32)
    b2 = np.asarray(b2, dtype=np.float32)

    B, S, Din = x.shape
    assert Din == D and gate_w.shape == (D, E)
    T = B * S
    xf = x.reshape(T, D)

    # ---- Host router (replicated gate): logits, top-2, softmax weights ----
    logits = xf.astype(np.float64) @ gate_w.astype(np.float64)  # [T, E]
    idx0 = np.argmax(logits, axis=1)
    rows = np.arange(T)
    v0 = logits[rows, idx0]
    l2 = logits.copy()
    l2[rows, idx0] = -np.inf
    idx1 = np.argmax(l2, axis=1)
    v1 = l2[rows, idx1]
    # softmax over the two top logits
    e1 = np.exp(v1 - v0)
    cw0 = 1.0 / (1.0 + e1)
    cw1 = e1 / (1.0 + e1)

    # ---- Dispatch: gather token ids per expert ----
    token_ids = []
    combine_w = []
    for e in range(E):
        sel0 = idx0 == e
        sel1 = idx1 == e
        ids = np.nonzero(sel0 | sel1)[0]
        w = np.where(sel0[ids], cw0[ids], cw1[ids])
        token_ids.append(ids)
        combine_w.append(w)

    max_n = max(len(ids) for ids in token_ids)
    C = max(CT, int(math.ceil(max_n / 128.0)) * 128)

    if C not in _NC_CACHE:
        _NC_CACHE[C] = _build(C)
    nc = _NC_CACHE[C]

    # ---- Build per-core input maps ----
    in_maps = []
    for e in range(E):
        ids = token_ids[e]
        n_e = len(ids)
        xT = np.zeros((KO, 128, C), dtype=BF16)
        if n_e:
            # [n_e, D] -> [D, n_e] -> [KO, 128, n_e]
            xg = xf[ids].T.reshape(KO, 128, n_e)
            xT[:, :, :n_e] = xg.astype(BF16)
        in_maps.append(
            {
                "xT": xT,
                "w1": np.ascontiguousarray(
                    w1[e].reshape(KO, 128, F).astype(BF16)
                ),
                "w2": np.ascontiguousarray(
                    w2[e].reshape(FO, 128, D).astype(BF16)
                ),
                "b1": np.ascontiguousarray(b1[e].reshape(FO, 128).T),
            }
        )

    res = run_bass_kernel_spmd(nc, in_maps, core_ids=list(range(N_CORES)))
    LAST_RESULTS = res

    # ---- Combine on host: out[t] += cw * (y_e[t] + b2[e]) ----
    out = np.zeros((T, D), dtype=np.float32)
    for e in range(E):
        ids = token_ids[e]
        n_e = len(ids)
        if n_e == 0:
            continue
        y_t = res.results[e]["yT"].reshape(D, C)[:, :n_e].T  # [n_e, D]
        out[ids] += combine_w[e][:, None].astype(np.float32) * (y_t + b2[e])

    return out.reshape(B, S, D)
